# revision 4
# baseline (speedup 1.0000x reference)
"""Causal single-head attention (S=4096, D=1024, fp32) on 8 TRN2 NeuronCores.

v7.1: uniform causal stripe decomposition (SPMD-friendly).
- Rows are 16-row stripes dealt round-robin to cores (stripe s -> core s%8),
  packed per-core in DESCENDING row order. For col-tile j (128 cols) the live
  rows are then exactly the first N_j = 512-16*j packed rows on EVERY core, so
  one instruction schedule serves all cores; only a single 16-row diagonal
  stripe per tile needs masking, via one per-core [128,16] 0/1 mask input.
- K/V projections are 8-way sharded: core c projects positions [512c,512c+512);
  two 8-way Shared-output AllGathers (position halves a/b, K^T+V packed)
  broadcast them. A tiny dummy collective at kernel start absorbs the
  collectives entry barrier; projections are emitted a-half first so AG_a
  triggers as early as possible.
- Scores / exp / row-sum / A@V stream over the gathered slabs; row sums
  accumulate in one PSUM bank across all tiles (ones-matmul); A@V accumulates
  per chunk-half in PSUM then adds into SBUF fp32 accumulators (alternating
  vector/gpsimd). Per-slot normalization is interleaved into pass b.
"""

import numpy as np
import ml_dtypes

import concourse.bacc as bacc
import concourse.tile as tile
from concourse import mybir
from concourse.bass_utils import run_bass_kernel_spmd

S = 4096
D = 1024
NCORES = 8
ROWS = 512
P = 128
DC = 8
BF = mybir.dt.bfloat16
F32 = mybir.dt.float32
EXP = mybir.ActivationFunctionType.Exp
ALL8 = [[0, 1, 2, 3, 4, 5, 6, 7]]

bf16 = ml_dtypes.bfloat16


def build_nc():
    nc = bacc.Bacc(None, target_bir_lowering=False, debug=False)

    xq = nc.declare_dram_parameter("xqt", [D, ROWS], BF, isOutput=False)
    xk = nc.declare_dram_parameter("xkt", [D, ROWS], BF, isOutput=False)
    xv = nc.declare_dram_parameter("xvt", [D, ROWS], BF, isOutput=False)
    wq = nc.declare_dram_parameter("wqt", [D, D], BF, isOutput=False)
    wk = nc.declare_dram_parameter("wkt", [D, D], BF, isOutput=False)
    wv = nc.declare_dram_parameter("wvt", [D, D], BF, isOutput=False)
    msk = nc.declare_dram_parameter("mk", [P, 16], BF, isOutput=False)
    out = nc.declare_dram_parameter("out", [ROWS, D], F32, isOutput=True)

    # kvin layout [128, 4096] bf16 (1MB): [:, 256*oc : 256*oc+256] = K^T d-chunk
    # oc for this half's 256 positions; [:, 2048 + 1024*jl + 512*ob : +512] =
    # V rows for local position-tile jl (0/1) and output half ob.
    kvin = [nc.dram_tensor(f"kvin{h}", [P, 4096], BF) for h in range(2)]
    kvout = [
        nc.dram_tensor(f"kvout{h}", [NCORES * P, 4096], BF, addr_space="Shared")
        for h in range(2)
    ]
    dumb_in = nc.dram_tensor("dumb_in", [1, 16], BF)
    dumb_out = nc.dram_tensor("dumb_out", [NCORES, 16], BF, addr_space="Shared")

    with tile.TileContext(nc) as tc:
        with (
            tc.tile_pool(name="persist", bufs=1) as persist,
            tc.tile_pool(name="kp", bufs=1) as kp,
            tc.tile_pool(name="stg", bufs=6) as stg,
            tc.tile_pool(name="kvs", bufs=3) as kvs,
            tc.tile_pool(name="att", bufs=6) as ap,
            tc.tile_pool(name="att_out", bufs=4) as op,
            tc.tile_pool(name="pps", bufs=3, space="PSUM") as pps,
            tc.tile_pool(name="avs", bufs=4, space="PSUM") as avsum,
            tc.tile_pool(name="ops", bufs=1, space="PSUM") as opsum,
        ):
            # dummy collective first: absorbs the collectives entry barrier
            dz = persist.tile([1, 16], BF, tag="dz", name="dz")
            nc.vector.memset(dz[:], 0.0)
            nc.gpsimd.dma_start(out=dumb_in[:, :], in_=dz[:])
            nc.gpsimd.collective_compute(
                "AllGather",
                mybir.AluOpType.bypass,
                replica_groups=ALL8,
                ins=[dumb_in[:].opt()],
                outs=[dumb_out[:].opt()],
            )

            ones = persist.tile([P, 16], BF, tag="ones", name="ones")
            nc.vector.memset(ones[:], 1.0)
            zbias = persist.tile([P, 1], F32, tag="zbias", name="zbias")
            nc.vector.memset(zbias[:], 0.0)
            mk_t = persist.tile([P, 16], BF, tag="mk", name="mk")
            nc.gpsimd.dma_start(out=mk_t[:], in_=msk[:, :])
            qT = [persist.tile([P, ROWS], BF, tag=f"qT{t}", name=f"qT{t}") for t in range(DC)]
            acc = {}
            for r in range(4):
                for ob in range(2):
                    acc[r, ob] = persist.tile([P, 512], F32, tag=f"acc{r}{ob}", name=f"acc{r}{ob}")
                    nc.vector.memset(acc[r, ob][:], 0.0)
            sums_bank = opsum.tile([P, 64], F32, tag="sums", name="sums")

            wk_t = [kp.tile([P, D], BF, tag=f"wk{d_}", name=f"wk{d_}") for d_ in range(DC)]
            wv_t = [kp.tile([P, D], BF, tag=f"wv{d_}", name=f"wv{d_}") for d_ in range(DC)]
            wq_t = [kp.tile([P, D], BF, tag=f"wq{d_}", name=f"wq{d_}") for d_ in range(DC)]
            xk_t = [kp.tile([P, ROWS], BF, tag=f"xk{d_}", name=f"xk{d_}") for d_ in range(DC)]
            xv_t = [kp.tile([P, ROWS], BF, tag=f"xv{d_}", name=f"xv{d_}") for d_ in range(DC)]
            xq_t = [kp.tile([P, ROWS], BF, tag=f"xq{d_}", name=f"xq{d_}") for d_ in range(DC)]

            for d_ in range(DC):
                nc.sync.dma_start(out=wk_t[d_][:], in_=wk[d_ * P:(d_ + 1) * P, :])
                nc.sync.dma_start(out=xk_t[d_][:], in_=xk[d_ * P:(d_ + 1) * P, :])
            for d_ in range(DC):
                nc.sync.dma_start(out=wv_t[d_][:], in_=wv[d_ * P:(d_ + 1) * P, :])
                nc.sync.dma_start(out=xv_t[d_][:], in_=xv[d_ * P:(d_ + 1) * P, :])

            def k_proj_half(h):
                for oc in range(DC):
                    ps = pps.tile([P, 512], F32, tag="pp", name="ppk")
                    for d_ in range(DC):
                        nc.tensor.matmul(
                            ps[:, 0:256],
                            lhsT=wk_t[d_][:, oc * P:(oc + 1) * P],
                            rhs=xk_t[d_][:, 256 * h:256 * h + 256],
                            start=(d_ == 0),
                            stop=(d_ == DC - 1),
                        )
                    sg = stg.tile([P, 256], BF, tag="sgk", name="sgk")
                    nc.scalar.copy(sg[:], ps[:, 0:256])
                    nc.gpsimd.dma_start(out=kvin[h][:, 256 * oc:256 * oc + 256], in_=sg[:])

            def v_proj_half(h):
                for jh in (2 * h, 2 * h + 1):
                    for ob in range(2):
                        ps = pps.tile([P, 512], F32, tag="pp", name="ppv")
                        for d_ in range(DC):
                            nc.tensor.matmul(
                                ps[:],
                                lhsT=xv_t[d_][:, jh * P:(jh + 1) * P],
                                rhs=wv_t[d_][:, ob * 512:(ob + 1) * 512],
                                start=(d_ == 0),
                                stop=(d_ == DC - 1),
                            )
                        sg = stg.tile([P, 512], BF, tag="sgv", name="sgv")
                        nc.scalar.copy(sg[:], ps[:])
                        off = 2048 + 1024 * (jh % 2) + 512 * ob
                        nc.gpsimd.dma_start(out=kvin[h][:, off:off + 512], in_=sg[:])

            def trigger_ag(h):
                nc.gpsimd.collective_compute(
                    "AllGather",
                    mybir.AluOpType.bypass,
                    replica_groups=ALL8,
                    ins=[kvin[h][:].opt()],
                    outs=[kvout[h][:].opt()],
                )

            k_proj_half(0)
            v_proj_half(0)
            trigger_ag(0)
            k_proj_half(1)
            v_proj_half(1)
            trigger_ag(1)

            # Q projection (loads emitted after AG triggers to keep DMA light
            # before the gathers; needed only ~45us in)
            for d_ in range(DC):
                nc.sync.dma_start(out=wq_t[d_][:], in_=wq[d_ * P:(d_ + 1) * P, :])
                nc.sync.dma_start(out=xq_t[d_][:], in_=xq[d_ * P:(d_ + 1) * P, :])
            for oc in range(DC):
                ps = pps.tile([P, 512], F32, tag="pp", name="ppq")
                for d_ in range(DC):
                    nc.tensor.matmul(
                        ps[:],
                        lhsT=wq_t[d_][:, oc * P:(oc + 1) * P],
                        rhs=xq_t[d_][:],
                        start=(d_ == 0),
                        stop=(d_ == DC - 1),
                    )
                nc.scalar.copy(qT[oc][:], ps[:])

            def normalize_slot(r):
                ssb = op.tile([P, 1], F32, tag="ssb", name="ssb")
                nc.vector.tensor_copy(ssb[:], sums_bank[:, 16 * r:16 * r + 1])
                rec = op.tile([P, 1], F32, tag=f"rec{r}", name=f"rec{r}")
                nc.vector.reciprocal(rec[:], ssb[:])
                for ob in range(2):
                    osb = op.tile([P, 512], F32, tag="osb", name="osb")
                    nc.vector.tensor_scalar_mul(osb[:], acc[r, ob][:], rec[:])
                    nc.sync.dma_start(
                        out=out[r * P:(r + 1) * P, ob * 512:(ob + 1) * 512], in_=osb[:]
                    )

            # ---- attention: stream gathered chunk-halves ----
            for h in range(2):
                for m in range(NCORES):
                    kv = kvs.tile([P, 4096], BF, tag="kv", name="kv")
                    nc.scalar.dma_start(out=kv[:], in_=kvout[h][m * P:(m + 1) * P, :])
                    pts = []
                    for jl in range(2):
                        j = 4 * m + 2 * h + jl
                        N = 512 - 16 * j
                        sp = pps.tile([P, 512], F32, tag="pp", name="sps")
                        for oc in range(DC):
                            nc.tensor.matmul(
                                sp[:, 0:N],
                                lhsT=kv[:, 256 * oc + 128 * jl:256 * oc + 128 * jl + 128],
                                rhs=qT[oc][:, 0:N],
                                start=(oc == 0),
                                stop=(oc == DC - 1),
                            )
                        pt = ap.tile([P, 512], BF, tag="pt", name="pt")
                        nc.scalar.activation(pt[:, 0:N], sp[:, 0:N], EXP, bias=zbias[:])
                        nc.vector.tensor_mul(pt[:, N - 16:N], pt[:, N - 16:N], mk_t[:])
                        pts.append((pt, N, jl))
                        for r in range((N + 127) // 128):
                            M = min(128, N - 128 * r)
                            nc.tensor.matmul(
                                sums_bank[0:M, 16 * r:16 * r + 16],
                                lhsT=pt[:, 128 * r:128 * r + M],
                                rhs=ones[:],
                                start=(j == 0),
                                stop=(j == 31 - 8 * r),
                                skip_group_check=True,
                            )
                    # A@V for this chunk-half, accumulated over its 2 tiles
                    N0 = pts[0][1]
                    for r in range((N0 + 127) // 128):
                        live = [(pt, min(128, N - 128 * r), jl)
                                for (pt, N, jl) in pts if N > 128 * r]
                        M0 = live[0][1]
                        for ob in range(2):
                            sc = avsum.tile([P, 512], F32, tag="avs", name="avs")
                            for i, (pt, M, jl) in enumerate(live):
                                nc.tensor.matmul(
                                    sc[0:M, :],
                                    lhsT=pt[:, 128 * r:128 * r + M],
                                    rhs=kv[:, 2048 + 1024 * jl + 512 * ob:
                                           2048 + 1024 * jl + 512 * ob + 512],
                                    start=(i == 0),
                                    stop=(i == len(live) - 1),
                                )
                            nc.vector.tensor_add(
                                acc[r, ob][0:M0, :], acc[r, ob][0:M0, :], sc[0:M0, :]
                            )
                    # normalize slot r right after its last sums tile lands
                    if h == 1 and m % 2 == 1:
                        normalize_slot((7 - m) // 2)
    return nc


_CACHE = {}


def _get_nc():
    if "nc" not in _CACHE:
        nc = build_nc()
        nc.compile()
        _CACHE["nc"] = nc
    return _CACHE["nc"]


def _rows_desc(c):
    return sorted([r for r in range(S) if (r // 16) % NCORES == c], reverse=True)


def build_in_maps(inputs):
    x_q = np.asarray(inputs["encodings_for_q"], dtype=np.float32)
    x_k = np.asarray(inputs["encodings_for_k"], dtype=np.float32)
    x_v = np.asarray(inputs["encodings_for_v"], dtype=np.float32)
    W_q = np.asarray(inputs["W_q"], dtype=np.float32)
    W_k = np.asarray(inputs["W_k"], dtype=np.float32)
    W_v = np.asarray(inputs["W_v"], dtype=np.float32)

    wqt = np.ascontiguousarray(W_q.T).astype(bf16)
    wkt = np.ascontiguousarray(W_k.T / np.sqrt(D)).astype(bf16)
    wvt = np.ascontiguousarray(W_v.T).astype(bf16)

    p_idx = np.arange(P)[:, None]
    t_idx = np.arange(16)[None, :]

    in_maps = []
    for c in range(NCORES):
        rows = _rows_desc(c)
        pos = slice(ROWS * c, ROWS * (c + 1))
        xqt_c = np.ascontiguousarray(x_q[rows].T).astype(bf16)
        xkt_c = np.ascontiguousarray(x_k[pos].T).astype(bf16)
        xvt_c = np.ascontiguousarray(x_v[pos].T).astype(bf16)
        mk_c = (p_idx <= 16 * c + 15 - t_idx).astype(bf16)
        in_maps.append(
            dict(
                xqt=xqt_c, xkt=xkt_c, xvt=xvt_c,
                wqt=wqt, wkt=wkt, wvt=wvt,
                mk=np.ascontiguousarray(mk_c),
            )
        )
    return in_maps


def kernel(**inputs):
    nc = _get_nc()
    in_maps = build_in_maps(inputs)
    res = run_bass_kernel_spmd(nc, in_maps, list(range(NCORES)))
    full = np.zeros((S, D), dtype=np.float32)
    for c in range(NCORES):
        full[_rows_desc(c)] = np.asarray(res.results[c]["out"], dtype=np.float32)
    return full


# revision 5
# speedup vs baseline: 1.3023x; 1.3023x over previous
"""Causal single-head attention (S=4096, D=1024, fp32) on 8 TRN2 NeuronCores.

v7.2: uniform causal stripe decomposition (SPMD-friendly).
- Rows are 16-row stripes dealt round-robin to cores (stripe s -> core s%8),
  packed per-core in DESCENDING row order. For col-tile j (128 cols) the live
  rows are then exactly the first N_j = 512-16*j packed rows on EVERY core, so
  one instruction schedule serves all cores; only a single 16-row diagonal
  stripe per tile needs masking, via one per-core [128,16] 0/1 mask input.
- K/V projections are 8-way sharded: core c projects positions [512c,512c+512).
  Four 8-way Shared-output AllGathers broadcast them, split by position half
  (a/b) and by K/V, pipelined against compute: K halves travel as fp8e4m3
  (half the wire; scores matmul runs mixed fp8 x bf16, with the 1/sqrt(D)
  scale folded into Q so K magnitudes stay in fp8's normal range), V halves
  as bf16.
- Scores / exp / row-sum / A@V stream over the gathered slabs; row sums
  accumulate in one PSUM bank across all tiles (ones-matmul); A@V accumulates
  per chunk-half in PSUM then vector-adds into SBUF fp32 accumulators.
  Per-slot normalization is interleaved into pass b.
"""

import numpy as np
import ml_dtypes

import concourse.bacc as bacc
import concourse.tile as tile
from concourse import mybir
from concourse.bass_utils import run_bass_kernel_spmd

S = 4096
D = 1024
NCORES = 8
ROWS = 512
P = 128
DC = 8
BF = mybir.dt.bfloat16
F8 = mybir.dt.float8e4
F32 = mybir.dt.float32
EXP = mybir.ActivationFunctionType.Exp
ALL8 = [[0, 1, 2, 3, 4, 5, 6, 7]]

bf16 = ml_dtypes.bfloat16
f8e4 = ml_dtypes.float8_e4m3fn


def build_nc():
    nc = bacc.Bacc(None, target_bir_lowering=False, debug=False)

    xq = nc.declare_dram_parameter("xqt", [D, ROWS], BF, isOutput=False)
    xk = nc.declare_dram_parameter("xkt", [D, ROWS], BF, isOutput=False)
    xv = nc.declare_dram_parameter("xvt", [D, ROWS], BF, isOutput=False)
    wq = nc.declare_dram_parameter("wqt", [D, D], BF, isOutput=False)
    wk = nc.declare_dram_parameter("wkt", [D, D], BF, isOutput=False)
    wv = nc.declare_dram_parameter("wvt", [D, D], BF, isOutput=False)
    msk = nc.declare_dram_parameter("mk", [P, 16], BF, isOutput=False)
    out = nc.declare_dram_parameter("out", [ROWS, D], F32, isOutput=True)

    # Per position-half h: K^T as fp8 [128, 8 d-chunks x 256 pos] and V as
    # bf16 [128, 2 pos-tiles x 2 out-halves x 512].
    kin = [nc.dram_tensor(f"kin{h}", [P, 2048], F8) for h in range(2)]
    kout = [
        nc.dram_tensor(f"kout{h}", [NCORES * P, 2048], F8, addr_space="Shared")
        for h in range(2)
    ]
    vin = [nc.dram_tensor(f"vin{h}", [P, 2048], BF) for h in range(2)]
    vout = [
        nc.dram_tensor(f"vout{h}", [NCORES * P, 2048], BF, addr_space="Shared")
        for h in range(2)
    ]

    with tile.TileContext(nc) as tc:
        with (
            tc.tile_pool(name="persist", bufs=1) as persist,
            tc.tile_pool(name="kp", bufs=1) as kp,
            tc.tile_pool(name="stg", bufs=6) as stg,
            tc.tile_pool(name="kvs", bufs=3) as kvs,
            tc.tile_pool(name="att", bufs=6) as ap,
            tc.tile_pool(name="att_out", bufs=4) as op,
            tc.tile_pool(name="pps", bufs=3, space="PSUM") as pps,
            tc.tile_pool(name="avs", bufs=4, space="PSUM") as avsum,
            tc.tile_pool(name="ops", bufs=1, space="PSUM") as opsum,
        ):
            ones = persist.tile([P, 16], BF, tag="ones", name="ones")
            nc.vector.memset(ones[:], 1.0)
            zbias = persist.tile([P, 1], F32, tag="zbias", name="zbias")
            nc.vector.memset(zbias[:], 0.0)
            mk_t = persist.tile([P, 16], BF, tag="mk", name="mk")
            nc.gpsimd.dma_start(out=mk_t[:], in_=msk[:, :])
            qT = [persist.tile([P, ROWS], BF, tag=f"qT{t}", name=f"qT{t}") for t in range(DC)]
            acc = {}
            for r in range(4):
                for ob in range(2):
                    acc[r, ob] = persist.tile([P, 512], F32, tag=f"acc{r}{ob}", name=f"acc{r}{ob}")
                    nc.vector.memset(acc[r, ob][:], 0.0)
            sums_bank = opsum.tile([P, 64], F32, tag="sums", name="sums")

            wk_t = [kp.tile([P, D], BF, tag=f"wk{d_}", name=f"wk{d_}") for d_ in range(DC)]
            wv_t = [kp.tile([P, D], BF, tag=f"wv{d_}", name=f"wv{d_}") for d_ in range(DC)]
            wq_t = [kp.tile([P, D], BF, tag=f"wq{d_}", name=f"wq{d_}") for d_ in range(DC)]
            xk_t = [kp.tile([P, ROWS], BF, tag=f"xk{d_}", name=f"xk{d_}") for d_ in range(DC)]
            xv_t = [kp.tile([P, ROWS], BF, tag=f"xv{d_}", name=f"xv{d_}") for d_ in range(DC)]
            xq_t = [kp.tile([P, ROWS], BF, tag=f"xq{d_}", name=f"xq{d_}") for d_ in range(DC)]

            for d_ in range(DC):
                nc.sync.dma_start(out=wk_t[d_][:], in_=wk[d_ * P:(d_ + 1) * P, :])
                nc.sync.dma_start(out=xk_t[d_][:], in_=xk[d_ * P:(d_ + 1) * P, :])
            for d_ in range(DC):
                nc.sync.dma_start(out=wv_t[d_][:], in_=wv[d_ * P:(d_ + 1) * P, :])
                nc.sync.dma_start(out=xv_t[d_][:], in_=xv[d_ * P:(d_ + 1) * P, :])

            def k_proj_half(h):
                for oc in range(DC):
                    ps = pps.tile([P, 512], F32, tag="pp", name="ppk")
                    for d_ in range(DC):
                        nc.tensor.matmul(
                            ps[:, 0:256],
                            lhsT=wk_t[d_][:, oc * P:(oc + 1) * P],
                            rhs=xk_t[d_][:, 256 * h:256 * h + 256],
                            start=(d_ == 0),
                            stop=(d_ == DC - 1),
                        )
                    sg = stg.tile([P, 256], F8, tag="sgk", name="sgk")
                    nc.scalar.copy(sg[:], ps[:, 0:256])
                    nc.gpsimd.dma_start(out=kin[h][:, 256 * oc:256 * oc + 256], in_=sg[:])

            def v_proj_half(h):
                for jh in (2 * h, 2 * h + 1):
                    for ob in range(2):
                        ps = pps.tile([P, 512], F32, tag="pp", name="ppv")
                        for d_ in range(DC):
                            nc.tensor.matmul(
                                ps[:],
                                lhsT=xv_t[d_][:, jh * P:(jh + 1) * P],
                                rhs=wv_t[d_][:, ob * 512:(ob + 1) * 512],
                                start=(d_ == 0),
                                stop=(d_ == DC - 1),
                            )
                        sg = stg.tile([P, 512], BF, tag="sgv", name="sgv")
                        nc.scalar.copy(sg[:], ps[:])
                        off = 1024 * (jh % 2) + 512 * ob
                        nc.gpsimd.dma_start(out=vin[h][:, off:off + 512], in_=sg[:])

            def trigger_ag(tin, tout):
                nc.gpsimd.collective_compute(
                    "AllGather",
                    mybir.AluOpType.bypass,
                    replica_groups=ALL8,
                    ins=[tin[:].opt()],
                    outs=[tout[:].opt()],
                )

            k_proj_half(0)
            trigger_ag(kin[0], kout[0])
            v_proj_half(0)
            trigger_ag(vin[0], vout[0])
            k_proj_half(1)
            trigger_ag(kin[1], kout[1])
            v_proj_half(1)
            trigger_ag(vin[1], vout[1])

            # Q projection (loads emitted after AG triggers; needed ~45us in).
            # 1/sqrt(D) scale lives in wqt (host side).
            for d_ in range(DC):
                nc.sync.dma_start(out=wq_t[d_][:], in_=wq[d_ * P:(d_ + 1) * P, :])
                nc.sync.dma_start(out=xq_t[d_][:], in_=xq[d_ * P:(d_ + 1) * P, :])
            for oc in range(DC):
                ps = pps.tile([P, 512], F32, tag="pp", name="ppq")
                for d_ in range(DC):
                    nc.tensor.matmul(
                        ps[:],
                        lhsT=wq_t[d_][:, oc * P:(oc + 1) * P],
                        rhs=xq_t[d_][:],
                        start=(d_ == 0),
                        stop=(d_ == DC - 1),
                    )
                nc.scalar.copy(qT[oc][:], ps[:])

            def normalize_slot(r):
                ssb = op.tile([P, 1], F32, tag="ssb", name="ssb")
                nc.vector.tensor_copy(ssb[:], sums_bank[:, 16 * r:16 * r + 1])
                rec = op.tile([P, 1], F32, tag=f"rec{r}", name=f"rec{r}")
                nc.vector.reciprocal(rec[:], ssb[:])
                for ob in range(2):
                    osb = op.tile([P, 512], F32, tag="osb", name="osb")
                    nc.vector.tensor_scalar_mul(osb[:], acc[r, ob][:], rec[:])
                    nc.sync.dma_start(
                        out=out[r * P:(r + 1) * P, ob * 512:(ob + 1) * 512], in_=osb[:]
                    )

            # ---- attention: stream gathered chunk-halves ----
            for h in range(2):
                for m in range(NCORES):
                    kvk = kvs.tile([P, 2048], F8, tag="kvk", name="kvk")
                    nc.scalar.dma_start(out=kvk[:], in_=kout[h][m * P:(m + 1) * P, :])
                    kvv = kvs.tile([P, 2048], BF, tag="kvv", name="kvv")
                    nc.scalar.dma_start(out=kvv[:], in_=vout[h][m * P:(m + 1) * P, :])
                    pts = []
                    for jl in range(2):
                        j = 4 * m + 2 * h + jl
                        N = 512 - 16 * j
                        sp = pps.tile([P, 512], F32, tag="pp", name="sps")
                        for oc in range(DC):
                            nc.tensor.matmul(
                                sp[:, 0:N],
                                lhsT=kvk[:, 256 * oc + 128 * jl:256 * oc + 128 * jl + 128],
                                rhs=qT[oc][:, 0:N],
                                start=(oc == 0),
                                stop=(oc == DC - 1),
                            )
                        pt = ap.tile([P, 512], BF, tag="pt", name="pt")
                        nc.scalar.activation(pt[:, 0:N], sp[:, 0:N], EXP, bias=zbias[:])
                        nc.vector.tensor_mul(pt[:, N - 16:N], pt[:, N - 16:N], mk_t[:])
                        pts.append((pt, N, jl))
                        for r in range((N + 127) // 128):
                            M = min(128, N - 128 * r)
                            nc.tensor.matmul(
                                sums_bank[0:M, 16 * r:16 * r + 16],
                                lhsT=pt[:, 128 * r:128 * r + M],
                                rhs=ones[:],
                                start=(j == 0),
                                stop=(j == 31 - 8 * r),
                                skip_group_check=True,
                            )
                    # A@V for this chunk-half, accumulated over its 2 tiles
                    N0 = pts[0][1]
                    for r in range((N0 + 127) // 128):
                        live = [(pt, min(128, N - 128 * r), jl)
                                for (pt, N, jl) in pts if N > 128 * r]
                        M0 = live[0][1]
                        for ob in range(2):
                            sc = avsum.tile([P, 512], F32, tag="avs", name="avs")
                            for i, (pt, M, jl) in enumerate(live):
                                nc.tensor.matmul(
                                    sc[0:M, :],
                                    lhsT=pt[:, 128 * r:128 * r + M],
                                    rhs=kvv[:, 1024 * jl + 512 * ob:
                                            1024 * jl + 512 * ob + 512],
                                    start=(i == 0),
                                    stop=(i == len(live) - 1),
                                )
                            nc.vector.tensor_add(
                                acc[r, ob][0:M0, :], acc[r, ob][0:M0, :], sc[0:M0, :]
                            )
                    # normalize slot r right after its last sums tile lands
                    if h == 1 and m % 2 == 1:
                        normalize_slot((7 - m) // 2)
    return nc


_CACHE = {}


def _get_nc():
    if "nc" not in _CACHE:
        nc = build_nc()
        nc.compile()
        _CACHE["nc"] = nc
    return _CACHE["nc"]


def _rows_desc(c):
    return sorted([r for r in range(S) if (r // 16) % NCORES == c], reverse=True)


def build_in_maps(inputs):
    x_q = np.asarray(inputs["encodings_for_q"], dtype=np.float32)
    x_k = np.asarray(inputs["encodings_for_k"], dtype=np.float32)
    x_v = np.asarray(inputs["encodings_for_v"], dtype=np.float32)
    W_q = np.asarray(inputs["W_q"], dtype=np.float32)
    W_k = np.asarray(inputs["W_k"], dtype=np.float32)
    W_v = np.asarray(inputs["W_v"], dtype=np.float32)

    # 1/sqrt(D) on Q (not K): keeps K in fp8e4m3's normal range
    wqt = np.ascontiguousarray(W_q.T / np.sqrt(D)).astype(bf16)
    wkt = np.ascontiguousarray(W_k.T).astype(bf16)
    wvt = np.ascontiguousarray(W_v.T).astype(bf16)

    p_idx = np.arange(P)[:, None]
    t_idx = np.arange(16)[None, :]

    in_maps = []
    for c in range(NCORES):
        rows = _rows_desc(c)
        pos = slice(ROWS * c, ROWS * (c + 1))
        xqt_c = np.ascontiguousarray(x_q[rows].T).astype(bf16)
        xkt_c = np.ascontiguousarray(x_k[pos].T).astype(bf16)
        xvt_c = np.ascontiguousarray(x_v[pos].T).astype(bf16)
        mk_c = (p_idx <= 16 * c + 15 - t_idx).astype(bf16)
        in_maps.append(
            dict(
                xqt=xqt_c, xkt=xkt_c, xvt=xvt_c,
                wqt=wqt, wkt=wkt, wvt=wvt,
                mk=np.ascontiguousarray(mk_c),
            )
        )
    return in_maps


def kernel(**inputs):
    nc = _get_nc()
    in_maps = build_in_maps(inputs)
    res = run_bass_kernel_spmd(nc, in_maps, list(range(NCORES)))
    full = np.zeros((S, D), dtype=np.float32)
    for c in range(NCORES):
        full[_rows_desc(c)] = np.asarray(res.results[c]["out"], dtype=np.float32)
    return full


# revision 6
# speedup vs baseline: 1.4584x; 1.1199x over previous
"""Causal single-head attention (S=4096, D=1024, fp32) on 8 TRN2 NeuronCores.

v7.3: uniform causal stripe decomposition (SPMD-friendly).
- Rows are 16-row stripes dealt round-robin to cores (stripe s -> core s%8),
  packed per-core in DESCENDING row order. For col-tile j (128 cols) the live
  rows are then exactly the first N_j = 512-16*j packed rows on EVERY core, so
  one instruction schedule serves all cores; only a single 16-row diagonal
  stripe per tile needs masking, via one per-core [128,16] 0/1 mask input.
- K/V projections are 8-way sharded: core c projects positions [512c,512c+512).
  TWO 8-way Shared-output AllGathers broadcast them: K^T as fp8e4m3 (scores
  run mixed fp8 x bf16; the 1/sqrt(D) scale is folded into Q so K magnitudes
  stay in fp8's normal range), V as bf16. The serial collective chain is the
  spine: K lands first and gates the scores phase; V lands ~45us later and
  gates the A@V phase.
- Emission is phase-ordered for the in-order PE: all score/exp/row-sum tiles
  (chunks 0..7, gated on K only), then all A@V (gated on V), with A@V psum
  accumulated across each chunk's 4 col-tiles before one vector-add into SBUF
  fp32 accumulators. Row sums accumulate in one PSUM bank across all tiles.
  Per-slot normalization is interleaved right after its last contribution.
"""

import numpy as np
import ml_dtypes

import concourse.bacc as bacc
import concourse.tile as tile
from concourse import mybir
from concourse.bass_utils import run_bass_kernel_spmd

S = 4096
D = 1024
NCORES = 8
ROWS = 512
P = 128
DC = 8
BF = mybir.dt.bfloat16
F8 = mybir.dt.float8e4
F32 = mybir.dt.float32
EXP = mybir.ActivationFunctionType.Exp
ALL8 = [[0, 1, 2, 3, 4, 5, 6, 7]]

bf16 = ml_dtypes.bfloat16


def build_nc():
    nc = bacc.Bacc(None, target_bir_lowering=False, debug=False)

    xq = nc.declare_dram_parameter("xqt", [D, ROWS], BF, isOutput=False)
    xk = nc.declare_dram_parameter("xkt", [D, ROWS], BF, isOutput=False)
    xv = nc.declare_dram_parameter("xvt", [D, ROWS], BF, isOutput=False)
    wq = nc.declare_dram_parameter("wqt", [D, D], BF, isOutput=False)
    wk = nc.declare_dram_parameter("wkt", [D, D], BF, isOutput=False)
    wv = nc.declare_dram_parameter("wvt", [D, D], BF, isOutput=False)
    msk = nc.declare_dram_parameter("mk", [P, 16], BF, isOutput=False)
    out = nc.declare_dram_parameter("out", [ROWS, D], F32, isOutput=True)

    # K^T fp8 [128, 8 d-chunks x 512 pos]; V bf16 [128, 4 pos-tiles x 2 x 512]
    kin = nc.dram_tensor("kin", [P, 4096], F8)
    kout = nc.dram_tensor("kout", [NCORES * P, 4096], F8, addr_space="Shared")
    vin = nc.dram_tensor("vin", [P, 4096], BF)
    vout = nc.dram_tensor("vout", [NCORES * P, 4096], BF, addr_space="Shared")

    with tile.TileContext(nc) as tc:
        with (
            tc.tile_pool(name="persist", bufs=1) as persist,
            tc.tile_pool(name="kp", bufs=1) as kp,
            tc.tile_pool(name="stg", bufs=6) as stg,
            tc.tile_pool(name="kks", bufs=3) as kks,
            tc.tile_pool(name="vvs", bufs=3) as vvs,
            tc.tile_pool(name="att", bufs=34) as ap,
            tc.tile_pool(name="att_out", bufs=4) as op,
            tc.tile_pool(name="pps", bufs=3, space="PSUM") as pps,
            tc.tile_pool(name="avs", bufs=4, space="PSUM") as avsum,
            tc.tile_pool(name="ops", bufs=1, space="PSUM") as opsum,
        ):
            ones = persist.tile([P, 16], BF, tag="ones", name="ones")
            nc.vector.memset(ones[:], 1.0)
            zbias = persist.tile([P, 1], F32, tag="zbias", name="zbias")
            nc.vector.memset(zbias[:], 0.0)
            mk_t = persist.tile([P, 16], BF, tag="mk", name="mk")
            nc.gpsimd.dma_start(out=mk_t[:], in_=msk[:, :])
            qT = [persist.tile([P, ROWS], BF, tag=f"qT{t}", name=f"qT{t}") for t in range(DC)]
            acc = {}
            for r in range(4):
                for ob in range(2):
                    acc[r, ob] = persist.tile([P, 512], F32, tag=f"acc{r}{ob}", name=f"acc{r}{ob}")
                    nc.vector.memset(acc[r, ob][:], 0.0)
            sums_bank = opsum.tile([P, 64], F32, tag="sums", name="sums")

            wk_t = [kp.tile([P, D], BF, tag=f"wk{d_}", name=f"wk{d_}") for d_ in range(DC)]
            wv_t = [kp.tile([P, D], BF, tag=f"wv{d_}", name=f"wv{d_}") for d_ in range(DC)]
            wq_t = [kp.tile([P, D], BF, tag=f"wq{d_}", name=f"wq{d_}") for d_ in range(DC)]
            xk_t = [kp.tile([P, ROWS], BF, tag=f"xk{d_}", name=f"xk{d_}") for d_ in range(DC)]
            xv_t = [kp.tile([P, ROWS], BF, tag=f"xv{d_}", name=f"xv{d_}") for d_ in range(DC)]
            xq_t = [kp.tile([P, ROWS], BF, tag=f"xq{d_}", name=f"xq{d_}") for d_ in range(DC)]

            for d_ in range(DC):
                nc.sync.dma_start(out=wk_t[d_][:], in_=wk[d_ * P:(d_ + 1) * P, :])
                nc.sync.dma_start(out=xk_t[d_][:], in_=xk[d_ * P:(d_ + 1) * P, :])
            for d_ in range(DC):
                nc.sync.dma_start(out=wv_t[d_][:], in_=wv[d_ * P:(d_ + 1) * P, :])
                nc.sync.dma_start(out=xv_t[d_][:], in_=xv[d_ * P:(d_ + 1) * P, :])

            # ---- K projection: K^T chunk [1024, 512] -> fp8 -> gather ----
            for oc in range(DC):
                ps = pps.tile([P, 512], F32, tag="pp", name="ppk")
                for d_ in range(DC):
                    nc.tensor.matmul(
                        ps[:],
                        lhsT=wk_t[d_][:, oc * P:(oc + 1) * P],
                        rhs=xk_t[d_][:],
                        start=(d_ == 0),
                        stop=(d_ == DC - 1),
                    )
                sg = stg.tile([P, 512], F8, tag="sgk", name="sgk")
                nc.scalar.copy(sg[:], ps[:])
                nc.gpsimd.dma_start(out=kin[:, 512 * oc:512 * oc + 512], in_=sg[:])
            nc.gpsimd.collective_compute(
                "AllGather",
                mybir.AluOpType.bypass,
                replica_groups=ALL8,
                ins=[kin[:].opt()],
                outs=[kout[:].opt()],
            )

            # ---- V projection: V chunk [512, 1024] -> bf16 -> gather ----
            for jh in range(4):
                for ob in range(2):
                    ps = pps.tile([P, 512], F32, tag="pp", name="ppv")
                    for d_ in range(DC):
                        nc.tensor.matmul(
                            ps[:],
                            lhsT=xv_t[d_][:, jh * P:(jh + 1) * P],
                            rhs=wv_t[d_][:, ob * 512:(ob + 1) * 512],
                            start=(d_ == 0),
                            stop=(d_ == DC - 1),
                        )
                    sg = stg.tile([P, 512], BF, tag="sgv", name="sgv")
                    nc.scalar.copy(sg[:], ps[:])
                    off = 1024 * jh + 512 * ob
                    nc.gpsimd.dma_start(out=vin[:, off:off + 512], in_=sg[:])
            nc.gpsimd.collective_compute(
                "AllGather",
                mybir.AluOpType.bypass,
                replica_groups=ALL8,
                ins=[vin[:].opt()],
                outs=[vout[:].opt()],
            )

            # ---- Q projection (scale 1/sqrt(D) folded into wqt host-side) ----
            for d_ in range(DC):
                nc.sync.dma_start(out=wq_t[d_][:], in_=wq[d_ * P:(d_ + 1) * P, :])
                nc.sync.dma_start(out=xq_t[d_][:], in_=xq[d_ * P:(d_ + 1) * P, :])
            for oc in range(DC):
                ps = pps.tile([P, 512], F32, tag="pp", name="ppq")
                for d_ in range(DC):
                    nc.tensor.matmul(
                        ps[:],
                        lhsT=wq_t[d_][:, oc * P:(oc + 1) * P],
                        rhs=xq_t[d_][:],
                        start=(d_ == 0),
                        stop=(d_ == DC - 1),
                    )
                nc.scalar.copy(qT[oc][:], ps[:])

            # ---- scores phase: all chunks, gated on K gather only ----
            pts = {}
            for m in range(NCORES):
                kvk = kks.tile([P, 4096], F8, tag="kvk", name="kvk")
                nc.scalar.dma_start(out=kvk[:], in_=kout[m * P:(m + 1) * P, :])
                for jl in range(4):
                    j = 4 * m + jl
                    N = 512 - 16 * j
                    sp = pps.tile([P, 512], F32, tag="pp", name="sps")
                    for oc in range(DC):
                        nc.tensor.matmul(
                            sp[:, 0:N],
                            lhsT=kvk[:, 512 * oc + 128 * jl:512 * oc + 128 * jl + 128],
                            rhs=qT[oc][:, 0:N],
                            start=(oc == 0),
                            stop=(oc == DC - 1),
                        )
                    pt = ap.tile([P, 512], BF, tag="pt", name="pt")
                    nc.scalar.activation(pt[:, 0:N], sp[:, 0:N], EXP, bias=zbias[:])
                    nc.vector.tensor_mul(pt[:, N - 16:N], pt[:, N - 16:N], mk_t[:])
                    pts[m, jl] = (pt, N)
                    for r in range((N + 127) // 128):
                        M = min(128, N - 128 * r)
                        nc.tensor.matmul(
                            sums_bank[0:M, 16 * r:16 * r + 16],
                            lhsT=pt[:, 128 * r:128 * r + M],
                            rhs=ones[:],
                            start=(j == 0),
                            stop=(j == 31 - 8 * r),
                            skip_group_check=True,
                        )

            def normalize_slot(r):
                ssb = op.tile([P, 1], F32, tag="ssb", name="ssb")
                nc.vector.tensor_copy(ssb[:], sums_bank[:, 16 * r:16 * r + 1])
                rec = op.tile([P, 1], F32, tag=f"rec{r}", name=f"rec{r}")
                nc.vector.reciprocal(rec[:], ssb[:])
                for ob in range(2):
                    osb = op.tile([P, 512], F32, tag="osb", name="osb")
                    nc.vector.tensor_scalar_mul(osb[:], acc[r, ob][:], rec[:])
                    nc.sync.dma_start(
                        out=out[r * P:(r + 1) * P, ob * 512:(ob + 1) * 512], in_=osb[:]
                    )

            # ---- A@V phase: gated on V gather; psum-accumulate 4 tiles/chunk ----
            for m in range(NCORES):
                kvv = vvs.tile([P, 4096], BF, tag="kvv", name="kvv")
                nc.sync.dma_start(out=kvv[:], in_=vout[m * P:(m + 1) * P, :])
                N0 = pts[m, 0][1]
                for r in range((N0 + 127) // 128):
                    live = [(pts[m, jl][0], min(128, pts[m, jl][1] - 128 * r), jl)
                            for jl in range(4) if pts[m, jl][1] > 128 * r]
                    M0 = live[0][1]
                    for ob in range(2):
                        sc = avsum.tile([P, 512], F32, tag="avs", name="avs")
                        for i, (pt, M, jl) in enumerate(live):
                            nc.tensor.matmul(
                                sc[0:M, :],
                                lhsT=pt[:, 128 * r:128 * r + M],
                                rhs=kvv[:, 1024 * jl + 512 * ob:
                                        1024 * jl + 512 * ob + 512],
                                start=(i == 0),
                                stop=(i == len(live) - 1),
                            )
                        nc.vector.tensor_add(
                            acc[r, ob][0:M0, :], acc[r, ob][0:M0, :], sc[0:M0, :]
                        )
                # acc slot r complete after chunk 7-2r (its last contributor)
                if m % 2 == 1:
                    normalize_slot((7 - m) // 2)
    return nc


_CACHE = {}


def _get_nc():
    if "nc" not in _CACHE:
        nc = build_nc()
        nc.compile()
        _CACHE["nc"] = nc
    return _CACHE["nc"]


def _rows_desc(c):
    return sorted([r for r in range(S) if (r // 16) % NCORES == c], reverse=True)


def build_in_maps(inputs):
    x_q = np.asarray(inputs["encodings_for_q"], dtype=np.float32)
    x_k = np.asarray(inputs["encodings_for_k"], dtype=np.float32)
    x_v = np.asarray(inputs["encodings_for_v"], dtype=np.float32)
    W_q = np.asarray(inputs["W_q"], dtype=np.float32)
    W_k = np.asarray(inputs["W_k"], dtype=np.float32)
    W_v = np.asarray(inputs["W_v"], dtype=np.float32)

    # 1/sqrt(D) on Q (not K): keeps K in fp8e4m3's normal range
    wqt = np.ascontiguousarray(W_q.T / np.sqrt(D)).astype(bf16)
    wkt = np.ascontiguousarray(W_k.T).astype(bf16)
    wvt = np.ascontiguousarray(W_v.T).astype(bf16)

    p_idx = np.arange(P)[:, None]
    t_idx = np.arange(16)[None, :]

    in_maps = []
    for c in range(NCORES):
        rows = _rows_desc(c)
        pos = slice(ROWS * c, ROWS * (c + 1))
        xqt_c = np.ascontiguousarray(x_q[rows].T).astype(bf16)
        xkt_c = np.ascontiguousarray(x_k[pos].T).astype(bf16)
        xvt_c = np.ascontiguousarray(x_v[pos].T).astype(bf16)
        mk_c = (p_idx <= 16 * c + 15 - t_idx).astype(bf16)
        in_maps.append(
            dict(
                xqt=xqt_c, xkt=xkt_c, xvt=xvt_c,
                wqt=wqt, wkt=wkt, wvt=wvt,
                mk=np.ascontiguousarray(mk_c),
            )
        )
    return in_maps


def kernel(**inputs):
    nc = _get_nc()
    in_maps = build_in_maps(inputs)
    res = run_bass_kernel_spmd(nc, in_maps, list(range(NCORES)))
    full = np.zeros((S, D), dtype=np.float32)
    for c in range(NCORES):
        full[_rows_desc(c)] = np.asarray(res.results[c]["out"], dtype=np.float32)
    return full


# revision 11
# speedup vs baseline: 1.5179x; 1.0408x over previous
"""Causal single-head attention (S=4096, D=1024, fp32) on 8 TRN2 NeuronCores.

v7.3: uniform causal stripe decomposition (SPMD-friendly).
- Rows are 16-row stripes dealt round-robin to cores (stripe s -> core s%8),
  packed per-core in DESCENDING row order. For col-tile j (128 cols) the live
  rows are then exactly the first N_j = 512-16*j packed rows on EVERY core, so
  one instruction schedule serves all cores; only a single 16-row diagonal
  stripe per tile needs masking, via one per-core [128,16] 0/1 mask input.
- K/V projections are 8-way sharded: core c projects positions [512c,512c+512).
  TWO 8-way Shared-output AllGathers broadcast them: K^T as fp8e4m3 (scores
  run mixed fp8 x bf16; the 1/sqrt(D) scale is folded into Q so K magnitudes
  stay in fp8's normal range), V as bf16. The serial collective chain is the
  spine: K lands first and gates the scores phase; V lands ~45us later and
  gates the A@V phase.
- Emission is phase-ordered for the in-order PE: all score/exp/row-sum tiles
  (chunks 0..7, gated on K only), then all A@V (gated on V), with A@V psum
  accumulated across each chunk's 4 col-tiles before one vector-add into SBUF
  fp32 accumulators. Row sums accumulate in one PSUM bank across all tiles.
  Per-slot normalization is interleaved right after its last contribution.
"""

import numpy as np
import ml_dtypes

import concourse.bacc as bacc
import concourse.tile as tile
from concourse import mybir
from concourse.bass_utils import run_bass_kernel_spmd

S = 4096
D = 1024
NCORES = 8
ROWS = 512
P = 128
DC = 8
BF = mybir.dt.bfloat16
F8 = mybir.dt.float8e4
F32 = mybir.dt.float32
EXP = mybir.ActivationFunctionType.Exp
ALL8 = [[0, 1, 2, 3, 4, 5, 6, 7]]

bf16 = ml_dtypes.bfloat16


def build_nc():
    nc = bacc.Bacc(None, target_bir_lowering=False, debug=False)

    xq = nc.declare_dram_parameter("xqt", [D, ROWS], BF, isOutput=False)
    xk = nc.declare_dram_parameter("xkt", [D, ROWS], BF, isOutput=False)
    xv = nc.declare_dram_parameter("xvt", [D, ROWS], BF, isOutput=False)
    wq = nc.declare_dram_parameter("wqt", [D, D], BF, isOutput=False)
    wk = nc.declare_dram_parameter("wkt", [D, D], BF, isOutput=False)
    wv = nc.declare_dram_parameter("wvt", [D, D], BF, isOutput=False)
    msk = nc.declare_dram_parameter("mk", [P, 16], BF, isOutput=False)
    out = nc.declare_dram_parameter("out", [ROWS, D], F32, isOutput=True)

    # K^T fp8 [128, 8 d-chunks x 512 pos]; V bf16 [128, 4 pos-tiles x 2 x 512]
    kin = nc.dram_tensor("kin", [P, 4096], F8)
    kout = nc.dram_tensor("kout", [NCORES * P, 4096], F8, addr_space="Shared")
    vin = nc.dram_tensor("vin", [P, 4096], BF)
    vout = nc.dram_tensor("vout", [NCORES * P, 4096], BF, addr_space="Shared")

    with tile.TileContext(nc) as tc:
        with (
            tc.tile_pool(name="persist", bufs=1) as persist,
            tc.tile_pool(name="kp", bufs=1) as kp,
            tc.tile_pool(name="stg", bufs=6) as stg,
            tc.tile_pool(name="kks", bufs=3) as kks,
            tc.tile_pool(name="vvs", bufs=3) as vvs,
            tc.tile_pool(name="att", bufs=34) as ap,
            tc.tile_pool(name="att_out", bufs=4) as op,
            tc.tile_pool(name="pps", bufs=3, space="PSUM") as pps,
            tc.tile_pool(name="avs", bufs=4, space="PSUM") as avsum,
            tc.tile_pool(name="ops", bufs=1, space="PSUM") as opsum,
        ):
            ones = persist.tile([P, 16], BF, tag="ones", name="ones")
            nc.vector.memset(ones[:], 1.0)
            zbias = persist.tile([P, 1], F32, tag="zbias", name="zbias")
            nc.vector.memset(zbias[:], 0.0)
            mk_t = persist.tile([P, 16], BF, tag="mk", name="mk")
            nc.gpsimd.dma_start(out=mk_t[:], in_=msk[:, :])
            qT = [persist.tile([P, ROWS], BF, tag=f"qT{t}", name=f"qT{t}") for t in range(DC)]
            acc = {}
            for r in range(4):
                for ob in range(2):
                    acc[r, ob] = persist.tile([P, 512], F32, tag=f"acc{r}{ob}", name=f"acc{r}{ob}")
                    nc.vector.memset(acc[r, ob][:], 0.0)
            sums_bank = opsum.tile([P, 64], F32, tag="sums", name="sums")

            wk_t = [kp.tile([P, D], BF, tag=f"wk{d_}", name=f"wk{d_}") for d_ in range(DC)]
            wv_t = [kp.tile([P, D], BF, tag=f"wv{d_}", name=f"wv{d_}") for d_ in range(DC)]
            wq_t = [kp.tile([P, D], BF, tag=f"wq{d_}", name=f"wq{d_}") for d_ in range(DC)]
            xk_t = [kp.tile([P, ROWS], BF, tag=f"xk{d_}", name=f"xk{d_}") for d_ in range(DC)]
            xv_t = [kp.tile([P, ROWS], BF, tag=f"xv{d_}", name=f"xv{d_}") for d_ in range(DC)]
            xq_t = [kp.tile([P, ROWS], BF, tag=f"xq{d_}", name=f"xq{d_}") for d_ in range(DC)]

            for d_ in range(DC):
                nc.sync.dma_start(out=wk_t[d_][:], in_=wk[d_ * P:(d_ + 1) * P, :])
                nc.sync.dma_start(out=xk_t[d_][:], in_=xk[d_ * P:(d_ + 1) * P, :])
            for d_ in range(DC):
                nc.sync.dma_start(out=wv_t[d_][:], in_=wv[d_ * P:(d_ + 1) * P, :])
                nc.sync.dma_start(out=xv_t[d_][:], in_=xv[d_ * P:(d_ + 1) * P, :])

            # ---- K projection: K^T chunk [1024, 512] -> fp8 -> gather ----
            for oc in range(DC):
                ps = pps.tile([P, 512], F32, tag="pp", name="ppk")
                for d_ in range(DC):
                    nc.tensor.matmul(
                        ps[:],
                        lhsT=wk_t[d_][:, oc * P:(oc + 1) * P],
                        rhs=xk_t[d_][:],
                        start=(d_ == 0),
                        stop=(d_ == DC - 1),
                    )
                sg = stg.tile([P, 512], F8, tag="sgk", name="sgk")
                nc.scalar.copy(sg[:], ps[:])
                nc.gpsimd.dma_start(out=kin[:, 512 * oc:512 * oc + 512], in_=sg[:])
            # Q loads early: the sync engine blocks on collective completions
            # below, so everything it must do beforehand is emitted first.
            for d_ in range(DC):
                nc.sync.dma_start(out=wq_t[d_][:], in_=wq[d_ * P:(d_ + 1) * P, :])
                nc.sync.dma_start(out=xq_t[d_][:], in_=xq[d_ * P:(d_ + 1) * P, :])
            nc.gpsimd.collective_compute(
                "AllGather",
                mybir.AluOpType.bypass,
                replica_groups=ALL8,
                ins=[kin[:].opt()],
                outs=[kout[:].opt()],
            )

            # ---- V projection: V chunk [512, 1024] -> bf16 -> gather ----
            for jh in range(4):
                for ob in range(2):
                    ps = pps.tile([P, 512], F32, tag="pp", name="ppv")
                    for d_ in range(DC):
                        nc.tensor.matmul(
                            ps[:],
                            lhsT=xv_t[d_][:, jh * P:(jh + 1) * P],
                            rhs=wv_t[d_][:, ob * 512:(ob + 1) * 512],
                            start=(d_ == 0),
                            stop=(d_ == DC - 1),
                        )
                    sg = stg.tile([P, 512], BF, tag="sgv", name="sgv")
                    nc.scalar.copy(sg[:], ps[:])
                    off = 1024 * jh + 512 * ob
                    nc.scalar.dma_start(out=vin[:, off:off + 512], in_=sg[:])
            nc.gpsimd.collective_compute(
                "AllGather",
                mybir.AluOpType.bypass,
                replica_groups=ALL8,
                ins=[vin[:].opt()],
                outs=[vout[:].opt()],
            )

            # ---- Q projection (scale 1/sqrt(D) folded into wqt host-side) ----
            for oc in range(DC):
                ps = pps.tile([P, 512], F32, tag="pp", name="ppq")
                for d_ in range(DC):
                    nc.tensor.matmul(
                        ps[:],
                        lhsT=wq_t[d_][:, oc * P:(oc + 1) * P],
                        rhs=xq_t[d_][:],
                        start=(d_ == 0),
                        stop=(d_ == DC - 1),
                    )
                nc.scalar.copy(qT[oc][:], ps[:])

            # ---- scores phase: all chunks, gated on K gather only ----
            pts = {}
            for m in range(NCORES):
                kvk = kks.tile([P, 4096], F8, tag="kvk", name="kvk")
                nc.scalar.dma_start(out=kvk[:], in_=kout[m * P:(m + 1) * P, :])
                for jl in range(4):
                    j = 4 * m + jl
                    N = 512 - 16 * j
                    sp = pps.tile([P, 512], F32, tag="pp", name="sps")
                    for oc in range(DC):
                        nc.tensor.matmul(
                            sp[:, 0:N],
                            lhsT=kvk[:, 512 * oc + 128 * jl:512 * oc + 128 * jl + 128],
                            rhs=qT[oc][:, 0:N],
                            start=(oc == 0),
                            stop=(oc == DC - 1),
                        )
                    pt = ap.tile([P, 512], BF, tag="pt", name="pt")
                    nc.scalar.activation(pt[:, 0:N], sp[:, 0:N], EXP, bias=zbias[:])
                    nc.vector.tensor_mul(pt[:, N - 16:N], pt[:, N - 16:N], mk_t[:])
                    pts[m, jl] = (pt, N)
                    for r in range((N + 127) // 128):
                        M = min(128, N - 128 * r)
                        nc.tensor.matmul(
                            sums_bank[0:M, 16 * r:16 * r + 16],
                            lhsT=pt[:, 128 * r:128 * r + M],
                            rhs=ones[:],
                            start=(j == 0),
                            stop=(j == 31 - 8 * r),
                            skip_group_check=True,
                        )

            def normalize_slot(r):
                ssb = op.tile([P, 1], F32, tag="ssb", name="ssb")
                nc.vector.tensor_copy(ssb[:], sums_bank[:, 16 * r:16 * r + 1])
                rec = op.tile([P, 1], F32, tag=f"rec{r}", name=f"rec{r}")
                nc.vector.reciprocal(rec[:], ssb[:])
                for ob in range(2):
                    osb = op.tile([P, 512], F32, tag="osb", name="osb")
                    nc.vector.tensor_scalar_mul(osb[:], acc[r, ob][:], rec[:])
                    nc.sync.dma_start(
                        out=out[r * P:(r + 1) * P, ob * 512:(ob + 1) * 512], in_=osb[:]
                    )

            # ---- A@V phase: gated on V gather; psum-accumulate 4 tiles/chunk ----
            for m in range(NCORES):
                kvv = vvs.tile([P, 4096], BF, tag="kvv", name="kvv")
                nc.sync.dma_start(out=kvv[:], in_=vout[m * P:(m + 1) * P, :])
                N0 = pts[m, 0][1]
                for r in range((N0 + 127) // 128):
                    live = [(pts[m, jl][0], min(128, pts[m, jl][1] - 128 * r), jl)
                            for jl in range(4) if pts[m, jl][1] > 128 * r]
                    M0 = live[0][1]
                    for ob in range(2):
                        sc = avsum.tile([P, 512], F32, tag="avs", name="avs")
                        for i, (pt, M, jl) in enumerate(live):
                            nc.tensor.matmul(
                                sc[0:M, :],
                                lhsT=pt[:, 128 * r:128 * r + M],
                                rhs=kvv[:, 1024 * jl + 512 * ob:
                                        1024 * jl + 512 * ob + 512],
                                start=(i == 0),
                                stop=(i == len(live) - 1),
                            )
                        nc.vector.tensor_add(
                            acc[r, ob][0:M0, :], acc[r, ob][0:M0, :], sc[0:M0, :]
                        )
                # acc slot r complete after chunk 7-2r (its last contributor)
                if m % 2 == 1:
                    normalize_slot((7 - m) // 2)
    return nc


_CACHE = {}


def _get_nc():
    if "nc" not in _CACHE:
        nc = build_nc()
        nc.compile()
        _CACHE["nc"] = nc
    return _CACHE["nc"]


def _rows_desc(c):
    return sorted([r for r in range(S) if (r // 16) % NCORES == c], reverse=True)


def build_in_maps(inputs):
    x_q = np.asarray(inputs["encodings_for_q"], dtype=np.float32)
    x_k = np.asarray(inputs["encodings_for_k"], dtype=np.float32)
    x_v = np.asarray(inputs["encodings_for_v"], dtype=np.float32)
    W_q = np.asarray(inputs["W_q"], dtype=np.float32)
    W_k = np.asarray(inputs["W_k"], dtype=np.float32)
    W_v = np.asarray(inputs["W_v"], dtype=np.float32)

    # 1/sqrt(D) on Q (not K): keeps K in fp8e4m3's normal range
    wqt = np.ascontiguousarray(W_q.T / np.sqrt(D)).astype(bf16)
    wkt = np.ascontiguousarray(W_k.T).astype(bf16)
    wvt = np.ascontiguousarray(W_v.T).astype(bf16)

    p_idx = np.arange(P)[:, None]
    t_idx = np.arange(16)[None, :]

    in_maps = []
    for c in range(NCORES):
        rows = _rows_desc(c)
        pos = slice(ROWS * c, ROWS * (c + 1))
        xqt_c = np.ascontiguousarray(x_q[rows].T).astype(bf16)
        xkt_c = np.ascontiguousarray(x_k[pos].T).astype(bf16)
        xvt_c = np.ascontiguousarray(x_v[pos].T).astype(bf16)
        mk_c = (p_idx <= 16 * c + 15 - t_idx).astype(bf16)
        in_maps.append(
            dict(
                xqt=xqt_c, xkt=xkt_c, xvt=xvt_c,
                wqt=wqt, wkt=wkt, wvt=wvt,
                mk=np.ascontiguousarray(mk_c),
            )
        )
    return in_maps


def kernel(**inputs):
    nc = _get_nc()
    in_maps = build_in_maps(inputs)
    res = run_bass_kernel_spmd(nc, in_maps, list(range(NCORES)))
    full = np.zeros((S, D), dtype=np.float32)
    for c in range(NCORES):
        full[_rows_desc(c)] = np.asarray(res.results[c]["out"], dtype=np.float32)
    return full


# revision 12
# speedup vs baseline: 1.5561x; 1.0251x over previous
"""Causal single-head attention (S=4096, D=1024, fp32) on 8 TRN2 NeuronCores.

v7.3: uniform causal stripe decomposition (SPMD-friendly).
- Rows are 16-row stripes dealt round-robin to cores (stripe s -> core s%8),
  packed per-core in DESCENDING row order. For col-tile j (128 cols) the live
  rows are then exactly the first N_j = 512-16*j packed rows on EVERY core, so
  one instruction schedule serves all cores; only a single 16-row diagonal
  stripe per tile needs masking, via one per-core [128,16] 0/1 mask input.
- K/V projections are 8-way sharded: core c projects positions [512c,512c+512).
  TWO 8-way Shared-output AllGathers broadcast them: K^T as fp8e4m3 (scores
  run mixed fp8 x bf16; the 1/sqrt(D) scale is folded into Q so K magnitudes
  stay in fp8's normal range), V as bf16. The serial collective chain is the
  spine: K lands first and gates the scores phase; V lands ~45us later and
  gates the A@V phase.
- Emission is phase-ordered for the in-order PE: all score/exp/row-sum tiles
  (chunks 0..7, gated on K only), then all A@V (gated on V), with A@V psum
  accumulated across each chunk's 4 col-tiles before one vector-add into SBUF
  fp32 accumulators. Row sums accumulate in one PSUM bank across all tiles.
  Per-slot normalization is interleaved right after its last contribution.
"""

import numpy as np
import ml_dtypes

import concourse.bacc as bacc
import concourse.tile as tile
from concourse import mybir
from concourse.bass_utils import run_bass_kernel_spmd

S = 4096
D = 1024
NCORES = 8
ROWS = 512
P = 128
DC = 8
BF = mybir.dt.bfloat16
F8 = mybir.dt.float8e4
F32 = mybir.dt.float32
EXP = mybir.ActivationFunctionType.Exp
ALL8 = [[0, 1, 2, 3, 4, 5, 6, 7]]

bf16 = ml_dtypes.bfloat16


def build_nc():
    nc = bacc.Bacc(None, target_bir_lowering=False, debug=False)

    xq = nc.declare_dram_parameter("xqt", [D, ROWS], BF, isOutput=False)
    xk = nc.declare_dram_parameter("xkt", [D, ROWS], BF, isOutput=False)
    xv = nc.declare_dram_parameter("xvt", [D, ROWS], BF, isOutput=False)
    wq = nc.declare_dram_parameter("wqt", [D, D], BF, isOutput=False)
    wk = nc.declare_dram_parameter("wkt", [D, D], BF, isOutput=False)
    wv = nc.declare_dram_parameter("wvt", [D, D], BF, isOutput=False)
    msk = nc.declare_dram_parameter("mk", [P, 16], BF, isOutput=False)
    out = nc.declare_dram_parameter("out", [ROWS, D], F32, isOutput=True)

    # K^T fp8 [128, 8 d-chunks x 512 pos]; V bf16 split by output half ob:
    # vin{ob} [128, 4 pos-tiles x 512] so A@V for ob=0 can start a gather early
    kin = nc.dram_tensor("kin", [P, 4096], F8)
    kout = nc.dram_tensor("kout", [NCORES * P, 4096], F8, addr_space="Shared")
    vin = [nc.dram_tensor(f"vin{ob}", [P, 2048], BF) for ob in range(2)]
    vout = [
        nc.dram_tensor(f"vout{ob}", [NCORES * P, 2048], BF, addr_space="Shared")
        for ob in range(2)
    ]

    with tile.TileContext(nc) as tc:
        with (
            tc.tile_pool(name="persist", bufs=1) as persist,
            tc.tile_pool(name="kp", bufs=1) as kp,
            tc.tile_pool(name="stg", bufs=6) as stg,
            tc.tile_pool(name="kks", bufs=3) as kks,
            tc.tile_pool(name="vvs", bufs=3) as vvs,
            tc.tile_pool(name="att", bufs=34) as ap,
            tc.tile_pool(name="att_out", bufs=4) as op,
            tc.tile_pool(name="pps", bufs=3, space="PSUM") as pps,
            tc.tile_pool(name="avs", bufs=4, space="PSUM") as avsum,
            tc.tile_pool(name="ops", bufs=1, space="PSUM") as opsum,
        ):
            ones = persist.tile([P, 16], BF, tag="ones", name="ones")
            nc.vector.memset(ones[:], 1.0)
            zbias = persist.tile([P, 1], F32, tag="zbias", name="zbias")
            nc.vector.memset(zbias[:], 0.0)
            mk_t = persist.tile([P, 16], BF, tag="mk", name="mk")
            nc.gpsimd.dma_start(out=mk_t[:], in_=msk[:, :])
            qT = [persist.tile([P, ROWS], BF, tag=f"qT{t}", name=f"qT{t}") for t in range(DC)]
            acc = {}
            for r in range(4):
                for ob in range(2):
                    acc[r, ob] = persist.tile([P, 512], F32, tag=f"acc{r}{ob}", name=f"acc{r}{ob}")
                    nc.vector.memset(acc[r, ob][:], 0.0)
            sums_bank = opsum.tile([P, 64], F32, tag="sums", name="sums")

            wk_t = [kp.tile([P, D], BF, tag=f"wk{d_}", name=f"wk{d_}") for d_ in range(DC)]
            wv_t = [kp.tile([P, D], BF, tag=f"wv{d_}", name=f"wv{d_}") for d_ in range(DC)]
            wq_t = [kp.tile([P, D], BF, tag=f"wq{d_}", name=f"wq{d_}") for d_ in range(DC)]
            xk_t = [kp.tile([P, ROWS], BF, tag=f"xk{d_}", name=f"xk{d_}") for d_ in range(DC)]
            xv_t = [kp.tile([P, ROWS], BF, tag=f"xv{d_}", name=f"xv{d_}") for d_ in range(DC)]
            xq_t = [kp.tile([P, ROWS], BF, tag=f"xq{d_}", name=f"xq{d_}") for d_ in range(DC)]

            for d_ in range(DC):
                nc.sync.dma_start(out=wk_t[d_][:], in_=wk[d_ * P:(d_ + 1) * P, :])
                nc.sync.dma_start(out=xk_t[d_][:], in_=xk[d_ * P:(d_ + 1) * P, :])
            for d_ in range(DC):
                nc.sync.dma_start(out=wv_t[d_][:], in_=wv[d_ * P:(d_ + 1) * P, :])
                nc.sync.dma_start(out=xv_t[d_][:], in_=xv[d_ * P:(d_ + 1) * P, :])

            # ---- K projection: K^T chunk [1024, 512] -> fp8 -> gather ----
            for oc in range(DC):
                ps = pps.tile([P, 512], F32, tag="pp", name="ppk")
                for d_ in range(DC):
                    nc.tensor.matmul(
                        ps[:],
                        lhsT=wk_t[d_][:, oc * P:(oc + 1) * P],
                        rhs=xk_t[d_][:],
                        start=(d_ == 0),
                        stop=(d_ == DC - 1),
                    )
                sg = stg.tile([P, 512], F8, tag="sgk", name="sgk")
                nc.scalar.copy(sg[:], ps[:])
                nc.gpsimd.dma_start(out=kin[:, 512 * oc:512 * oc + 512], in_=sg[:])
            # Q loads early: the sync engine blocks on collective completions
            # below, so everything it must do beforehand is emitted first.
            for d_ in range(DC):
                nc.sync.dma_start(out=wq_t[d_][:], in_=wq[d_ * P:(d_ + 1) * P, :])
                nc.sync.dma_start(out=xq_t[d_][:], in_=xq[d_ * P:(d_ + 1) * P, :])
            nc.gpsimd.collective_compute(
                "AllGather",
                mybir.AluOpType.bypass,
                replica_groups=ALL8,
                ins=[kin[:].opt()],
                outs=[kout[:].opt()],
            )

            # ---- V projection: V chunk [512, 1024] -> bf16 -> gather ----
            for jh in range(4):
                for ob in range(2):
                    ps = pps.tile([P, 512], F32, tag="pp", name="ppv")
                    for d_ in range(DC):
                        nc.tensor.matmul(
                            ps[:],
                            lhsT=xv_t[d_][:, jh * P:(jh + 1) * P],
                            rhs=wv_t[d_][:, ob * 512:(ob + 1) * 512],
                            start=(d_ == 0),
                            stop=(d_ == DC - 1),
                        )
                    sg = stg.tile([P, 512], BF, tag="sgv", name="sgv")
                    nc.scalar.copy(sg[:], ps[:])
                    off = 512 * jh
                    nc.scalar.dma_start(out=vin[ob][:, off:off + 512], in_=sg[:])
            for ob in range(2):
                nc.gpsimd.collective_compute(
                    "AllGather",
                    mybir.AluOpType.bypass,
                    replica_groups=ALL8,
                    ins=[vin[ob][:].opt()],
                    outs=[vout[ob][:].opt()],
                )

            # ---- Q projection (scale 1/sqrt(D) folded into wqt host-side) ----
            for oc in range(DC):
                ps = pps.tile([P, 512], F32, tag="pp", name="ppq")
                for d_ in range(DC):
                    nc.tensor.matmul(
                        ps[:],
                        lhsT=wq_t[d_][:, oc * P:(oc + 1) * P],
                        rhs=xq_t[d_][:],
                        start=(d_ == 0),
                        stop=(d_ == DC - 1),
                    )
                nc.scalar.copy(qT[oc][:], ps[:])

            # ---- scores phase: all chunks, gated on K gather only ----
            pts = {}
            for m in range(NCORES):
                kvk = kks.tile([P, 4096], F8, tag="kvk", name="kvk")
                nc.scalar.dma_start(out=kvk[:], in_=kout[m * P:(m + 1) * P, :])
                for jl in range(4):
                    j = 4 * m + jl
                    N = 512 - 16 * j
                    sp = pps.tile([P, 512], F32, tag="pp", name="sps")
                    for oc in range(DC):
                        nc.tensor.matmul(
                            sp[:, 0:N],
                            lhsT=kvk[:, 512 * oc + 128 * jl:512 * oc + 128 * jl + 128],
                            rhs=qT[oc][:, 0:N],
                            start=(oc == 0),
                            stop=(oc == DC - 1),
                        )
                    pt = ap.tile([P, 512], BF, tag="pt", name="pt")
                    nc.scalar.activation(pt[:, 0:N], sp[:, 0:N], EXP, bias=zbias[:])
                    nc.vector.tensor_mul(pt[:, N - 16:N], pt[:, N - 16:N], mk_t[:])
                    pts[m, jl] = (pt, N)
                    for r in range((N + 127) // 128):
                        M = min(128, N - 128 * r)
                        nc.tensor.matmul(
                            sums_bank[0:M, 16 * r:16 * r + 16],
                            lhsT=pt[:, 128 * r:128 * r + M],
                            rhs=ones[:],
                            start=(j == 0),
                            stop=(j == 31 - 8 * r),
                            skip_group_check=True,
                        )

            def normalize_slot(r):
                ssb = op.tile([P, 1], F32, tag="ssb", name="ssb")
                nc.vector.tensor_copy(ssb[:], sums_bank[:, 16 * r:16 * r + 1])
                rec = op.tile([P, 1], F32, tag=f"rec{r}", name=f"rec{r}")
                nc.vector.reciprocal(rec[:], ssb[:])
                for ob in range(2):
                    osb = op.tile([P, 512], F32, tag="osb", name="osb")
                    nc.vector.tensor_scalar_mul(osb[:], acc[r, ob][:], rec[:])
                    nc.sync.dma_start(
                        out=out[r * P:(r + 1) * P, ob * 512:(ob + 1) * 512], in_=osb[:]
                    )

            # ---- A@V phase: ob-half at a time, gated on its V gather ----
            for ob in range(2):
                for m in range(NCORES):
                    kvv = vvs.tile([P, 2048], BF, tag="kvv", name="kvv")
                    nc.sync.dma_start(out=kvv[:], in_=vout[ob][m * P:(m + 1) * P, :])
                    N0 = pts[m, 0][1]
                    for r in range((N0 + 127) // 128):
                        live = [(pts[m, jl][0], min(128, pts[m, jl][1] - 128 * r), jl)
                                for jl in range(4) if pts[m, jl][1] > 128 * r]
                        M0 = live[0][1]
                        sc = avsum.tile([P, 512], F32, tag="avs", name="avs")
                        for i, (pt, M, jl) in enumerate(live):
                            nc.tensor.matmul(
                                sc[0:M, :],
                                lhsT=pt[:, 128 * r:128 * r + M],
                                rhs=kvv[:, 512 * jl:512 * jl + 512],
                                start=(i == 0),
                                stop=(i == len(live) - 1),
                            )
                        nc.vector.tensor_add(
                            acc[r, ob][0:M0, :], acc[r, ob][0:M0, :], sc[0:M0, :]
                        )
                    # acc slot r complete after ob=1 chunk 7-2r (last contributor)
                    if ob == 1 and m % 2 == 1:
                        normalize_slot((7 - m) // 2)
    return nc


_CACHE = {}


def _get_nc():
    if "nc" not in _CACHE:
        nc = build_nc()
        nc.compile()
        _CACHE["nc"] = nc
    return _CACHE["nc"]


def _rows_desc(c):
    return sorted([r for r in range(S) if (r // 16) % NCORES == c], reverse=True)


def build_in_maps(inputs):
    x_q = np.asarray(inputs["encodings_for_q"], dtype=np.float32)
    x_k = np.asarray(inputs["encodings_for_k"], dtype=np.float32)
    x_v = np.asarray(inputs["encodings_for_v"], dtype=np.float32)
    W_q = np.asarray(inputs["W_q"], dtype=np.float32)
    W_k = np.asarray(inputs["W_k"], dtype=np.float32)
    W_v = np.asarray(inputs["W_v"], dtype=np.float32)

    # 1/sqrt(D) on Q (not K): keeps K in fp8e4m3's normal range
    wqt = np.ascontiguousarray(W_q.T / np.sqrt(D)).astype(bf16)
    wkt = np.ascontiguousarray(W_k.T).astype(bf16)
    wvt = np.ascontiguousarray(W_v.T).astype(bf16)

    p_idx = np.arange(P)[:, None]
    t_idx = np.arange(16)[None, :]

    in_maps = []
    for c in range(NCORES):
        rows = _rows_desc(c)
        pos = slice(ROWS * c, ROWS * (c + 1))
        xqt_c = np.ascontiguousarray(x_q[rows].T).astype(bf16)
        xkt_c = np.ascontiguousarray(x_k[pos].T).astype(bf16)
        xvt_c = np.ascontiguousarray(x_v[pos].T).astype(bf16)
        mk_c = (p_idx <= 16 * c + 15 - t_idx).astype(bf16)
        in_maps.append(
            dict(
                xqt=xqt_c, xkt=xkt_c, xvt=xvt_c,
                wqt=wqt, wkt=wkt, wvt=wvt,
                mk=np.ascontiguousarray(mk_c),
            )
        )
    return in_maps


def kernel(**inputs):
    nc = _get_nc()
    in_maps = build_in_maps(inputs)
    res = run_bass_kernel_spmd(nc, in_maps, list(range(NCORES)))
    full = np.zeros((S, D), dtype=np.float32)
    for c in range(NCORES):
        full[_rows_desc(c)] = np.asarray(res.results[c]["out"], dtype=np.float32)
    return full
